# revision 4
# baseline (speedup 1.0000x reference)
"""Sparse-gather Trainium2 kernel v12 (row-split pipeline: epilogue+out of half A overlap gather of half B) for nn_BITypeNetwork.

Math (adj has exactly two ones per row):
    out = t1 + sbit*(1-t1),  t1 = c*x3,  sbit = states[j1]*states[j2]

Per core (2048 rows), TWO input DMA instructions total:
  * A u16 [128, 1568]: packed-states table (1024 words) | wrapped gather
    indices (32) | per-slot bit masks (512). Replicated per partition as
    needed; 392 KiB.
  * CX bf16 [128, 2, 256]: c and x3 per row, replicated across each
    16-partition gpsimd group; 128 KiB.
GPSIMD indirect_copy gathers the u16 word holding states[j] for 4096
slots (8 gpsimd cores x 512); DVE decodes (and, mult, is_gt) and runs
the epilogue; 8 KiB strided DMA out.

The backend inserts a chip-wide DMA-drain before the custom GPSIMD op;
it blocks while any DMA runs anywhere on the chip, and the 8 SPMD cores
start ~2-3us apart. Keeping each core's DMA busy window under ~2us
(few instructions, few bytes) lets every core's drain find an idle gap.
c is exact in bf16; x3 rounds at ~2^-9 relative (gate is 2e-2).
"""

import os
import sys

for _p in ("/opt/trn_rl_repo", "/opt/pypackages"):
    if os.path.isdir(_p) and _p not in sys.path:
        sys.path.insert(0, _p)

from contextlib import ExitStack

import ml_dtypes
import numpy as np

import concourse.bass as bass
import concourse.tile as tile
from concourse import bacc, mybir
from concourse.bass_utils import run_bass_kernel_spmd

N = 16384
CORES = 8
R = N // CORES        # 2048 rows per core
P = 128
G = 8
RG = R // G           # 256 rows per group
NV = 2 * RG           # 512 gather slots per group
S = NV // 16          # 32 wrapped u16 index slots per partition
TW = N // 16          # 1024 u16 words of packed states
AW = TW + S + NV      # 1568 u16 words per partition

U16 = mybir.dt.uint16
BF16 = mybir.dt.bfloat16
F32 = mybir.dt.float32


def build_nc():
    nc = bacc.Bacc()
    a_in = nc.declare_dram_parameter("a", [P, AW], U16, isOutput=False)
    cx_in = nc.declare_dram_parameter("cx", [P, 2, RG], BF16, isOutput=False)
    out = nc.declare_dram_parameter("out", [G, RG], F32, isOutput=True)

    mult = mybir.AluOpType.mult
    add = mybir.AluOpType.add
    band = mybir.AluOpType.bitwise_and
    isgt = mybir.AluOpType.is_gt

    with ExitStack() as ctx:
        tc = ctx.enter_context(tile.TileContext(nc))
        pool = ctx.enter_context(tc.tile_pool(name="main", bufs=1))

        a_t = pool.tile([P, AW], U16, tag="a")
        cx_t = pool.tile([P, 2, RG], BF16, tag="cx")
        nc.sync.dma_start(a_t[:], a_in[:, :])
        nc.scalar.dma_start(cx_t[:], cx_in[:, :, :])

        # t1/u depend only on cx: scheduled before the gather lands.
        t1 = pool.tile([P, RG], F32, tag="t1")
        nc.vector.tensor_tensor(t1[:], cx_t[:, 0, :], cx_t[:, 1, :], op=mult)
        u_t = pool.tile([P, RG], F32, tag="u")
        nc.vector.tensor_scalar(u_t[:], t1[:], -1.0, 1.0, op0=mult, op1=add)

        # Row-split pipeline: half A = rows 0:128 per group (slots: j1 then
        # j2), half B = rows 128:256.  Half A's decode, epilogue, and output
        # DMA all run while half B's gather executes on GPSIMD.
        H = RG // 2                     # 128 rows per half per group
        g_t = pool.tile([P, NV], U16, tag="g")
        andv = pool.tile([P, NV], U16, tag="andv")
        prod = pool.tile([P, RG], F32, tag="prod")
        sbit = pool.tile([P, RG], F32, tag="sbit")
        v_t = pool.tile([P, RG], F32, tag="v")
        res = pool.tile([P, RG], F32, tag="res")
        for h in range(2):
            lo, hi = h * RG, (h + 1) * RG          # slot block of this half
            r0, r1 = h * H, (h + 1) * H            # row block of this half
            nc.gpsimd.indirect_copy(
                g_t[:, lo:hi], a_t[:, 0:TW],
                a_t[:, TW + h * (S // 2) : TW + (h + 1) * (S // 2)], True,
            )
            nc.vector.tensor_tensor(
                andv[:, lo:hi], g_t[:, lo:hi], a_t[:, TW + S + lo : TW + S + hi],
                op=band,
            )
            nc.vector.tensor_tensor(
                prod[:, r0:r1], andv[:, lo : lo + H], andv[:, lo + H : hi], op=mult
            )
            nc.vector.tensor_scalar(sbit[:, r0:r1], prod[:, r0:r1], 0.0, None, op0=isgt)
            nc.vector.tensor_tensor(v_t[:, r0:r1], sbit[:, r0:r1], u_t[:, r0:r1], op=mult)
            nc.vector.tensor_tensor(res[:, r0:r1], t1[:, r0:r1], v_t[:, r0:r1], op=add)
            nc.sync.dma_start(out[:, r0:r1], res[0:P:16, r0:r1])

    nc.compile()
    return nc


_NC_CACHE = {}


def _get_nc():
    if "v12" not in _NC_CACHE:
        _NC_CACHE["v12"] = build_nc()
    return _NC_CACHE["v12"]


def prep_in_maps(x, adj, states, c):
    x = np.asarray(x, dtype=np.float32).reshape(-1)
    adj = np.asarray(adj, dtype=np.float32)
    states = np.asarray(states, dtype=np.float32).reshape(-1)
    c = np.asarray(c, dtype=np.float32).reshape(-1)
    x3 = np.roll(x, -1)

    rows_nz, cols_nz = np.nonzero(adj)
    assert rows_nz.shape[0] == 2 * N
    assert np.all(rows_nz.reshape(N, 2)[:, 0] == np.arange(N))
    jj = cols_nz.reshape(N, 2)

    sbits = (states != 0.0).astype(np.uint8)
    tblrow = np.packbits(sbits, bitorder="little").view("<u2")   # [1024] u16

    in_maps = []
    for m in range(CORES):
        j = jj[m * R : (m + 1) * R]
        cm = c[m * R : (m + 1) * R].astype(ml_dtypes.bfloat16)   # exact 0/1
        xm = x3[m * R : (m + 1) * R].astype(ml_dtypes.bfloat16)  # ~2^-9 rel

        a_arr = np.zeros((P, AW), dtype=np.uint16)
        a_arr[:, 0:TW] = tblrow[None, :]
        cx_arr = np.zeros((P, 2, RG), dtype=ml_dtypes.bfloat16)
        for g in range(G):
            jg = j[g * RG : (g + 1) * RG]
            unwrapped = np.concatenate(
                [jg[0:128, 0], jg[0:128, 1], jg[128:256, 0], jg[128:256, 1]]
            )     # [512]
            wordidx = (unwrapped >> 4).astype(np.uint16)
            bitmask = (np.uint16(1) << (unwrapped & 15).astype(np.uint16))
            lo = 16 * g
            a_arr[lo : lo + 16, TW : TW + S] = wordidx.reshape(S, 16).T
            a_arr[lo : lo + 16, TW + S : AW] = bitmask[None, :]
            cx_arr[lo : lo + 16, 0, :] = cm[g * RG : (g + 1) * RG][None, :]
            cx_arr[lo : lo + 16, 1, :] = xm[g * RG : (g + 1) * RG][None, :]
        in_maps.append({"a": a_arr, "cx": cx_arr})
    return in_maps


def _ensure_ntff_hook():
    import types

    try:
        from antenv.axon_hooks import get_axon_ntff_profile_hook  # noqa: F401

        return
    except ImportError:
        pass
    import antenv
    from trn_agent_boot.trn_boot import _ntff_profile_via_ctypes

    hook = _ntff_profile_via_ctypes("/opt/axon/libaxon_pjrt.so")
    mod = types.ModuleType("antenv.axon_hooks")
    state = {"hook": hook}
    mod.set_axon_ntff_profile_hook = lambda h: state.__setitem__("hook", h)
    mod.get_axon_ntff_profile_hook = lambda: state["hook"]
    sys.modules["antenv.axon_hooks"] = mod
    antenv.axon_hooks = mod


def run(x, adj, states, c, trace=False, **kw):
    if trace:
        _ensure_ntff_hook()
    in_maps = prep_in_maps(x, adj, states, c)
    nc = _get_nc()
    res = run_bass_kernel_spmd(nc, in_maps, list(range(CORES)), trace=trace, **kw)
    outs = []
    for m in range(CORES):
        o = np.asarray(res.results[m]["out"], dtype=np.float32)
        outs.append(o.reshape(R))
    return np.concatenate(outs), res


def kernel(x, adj, states, c):
    full, _ = run(x, adj, states, c)
    return full
